# revision 11
# baseline (speedup 1.0000x reference)
"""CIF layer kernel for TRN2 — 8-core data-parallel over batch.

Self-contained: hardcodes B=16, T=4096, D=512, U=128; 8 cores x 2 examples.
Per example: load x tiles [128t,512d] (f32r) -> PE transposes -> xT windows
(D-part) -> PE diag-matmul conv -> ACT/DVE relu -> PE matvec -> sigmoid ->
mask/sum/rescale -> DVE sequential scans (exact f32 recurrence) -> fires;
mass-split weights from clamped cumsum -> PE segment matmul -> acoustic.
"""
import ml_dtypes
import numpy as np
from contextlib import ExitStack

import concourse.bacc as bacc
import concourse.bass as bass
import concourse.mybir as mybir
import concourse.tile as tile
from concourse.bass_utils import run_bass_kernel_spmd

F32 = mybir.dt.float32
F32R = mybir.dt.float32r
BF16 = mybir.dt.bfloat16
I32 = mybir.dt.int32
AL = mybir.AluOpType
AF = mybir.ActivationFunctionType

B, T, D, U = 16, 4096, 512, 128
NCORES = 8
BPC = B // NCORES          # examples per core
NT = T // 128              # 32 t-tiles / example
NC4 = D // 128             # 4 d-chunks
NW = T // 512              # 8 windows of 512 t

_NC_CACHE = {}


def _build_nc():
    if "nc" in _NC_CACHE:
        return _NC_CACHE["nc"]
    nc = bacc.Bacc("TRN2", target_bir_lowering=False, debug=False)

    x_in = nc.dram_tensor("x", [BPC, T, D], F32R, kind="ExternalInput")
    xb_in = nc.dram_tensor("xb", [BPC, T, D], BF16, kind="ExternalInput")
    mask_in = nc.dram_tensor("mask", [BPC, T], F32, kind="ExternalInput")
    tgt_in = nc.dram_tensor("tgt", [BPC, 1], F32, kind="ExternalInput")
    diags_in = nc.dram_tensor("diags", [128, 12 * 128], F32R, kind="ExternalInput")
    ident_in = nc.dram_tensor("ident", [128, 128], F32R, kind="ExternalInput")
    linw_in = nc.dram_tensor("linw", [128, NC4], F32R, kind="ExternalInput")
    biasc_in = nc.dram_tensor("biasc", [128, NC4], F32, kind="ExternalInput")
    negj_in = nc.dram_tensor("negj", [128, 128], F32, kind="ExternalInput")

    ac_o = nc.dram_tensor("acoustic", [BPC, U, D], F32, kind="ExternalOutput")
    fires_o = nc.dram_tensor("fires", [BPC, T], F32, kind="ExternalOutput")
    alphas_o = nc.dram_tensor("alphas", [BPC, T], F32, kind="ExternalOutput")
    tnh_o = nc.dram_tensor("tnh", [BPC, 1], F32, kind="ExternalOutput")
    cscr = nc.dram_tensor("cscr", [BPC, T + 1], F32, kind="Internal")

    with tile.TileContext(nc) as tc, ExitStack() as ctx:
        cpool = ctx.enter_context(tc.tile_pool(name="consts", bufs=1))
        xpool = ctx.enter_context(tc.tile_pool(name="xp", bufs=1))
        xbpool = ctx.enter_context(tc.tile_pool(name="xb", bufs=1))
        wpool = ctx.enter_context(tc.tile_pool(name="win", bufs=3))
        hpool = ctx.enter_context(tc.tile_pool(name="hp", bufs=3))
        spool = ctx.enter_context(tc.tile_pool(name="small", bufs=1))
        kpool = ctx.enter_context(tc.tile_pool(name="chunk", bufs=2))
        wbpool = ctx.enter_context(tc.tile_pool(name="wb", bufs=1))
        ps_t = ctx.enter_context(tc.tile_pool(name="ps_t", bufs=2, space="PSUM"))
        ps_c = ctx.enter_context(tc.tile_pool(name="ps_c", bufs=2, space="PSUM"))
        ps_l = ctx.enter_context(tc.tile_pool(name="ps_l", bufs=2, space="PSUM"))
        ps_a = ctx.enter_context(tc.tile_pool(name="ps_a", bufs=2, space="PSUM"))

        diags = cpool.tile([128, 12 * 128], F32R)
        ident = cpool.tile([128, 128], F32R)
        linw = cpool.tile([128, NC4], F32R)
        biasc = cpool.tile([128, NC4], F32)
        negj = cpool.tile([128, 128], F32)
        mask2 = cpool.tile([BPC, T], F32)
        tgt2 = cpool.tile([BPC, 1], F32)
        for t_, src in [(diags, diags_in), (ident, ident_in), (linw, linw_in),
                        (biasc, biasc_in), (negj, negj_in), (mask2, mask_in),
                        (tgt2, tgt_in)]:
            nc.sync.dma_start(t_[:], src[:])

        sig2 = spool.tile([BPC, T], F32)        # sigmoid -> masked -> alphas
        chatp = spool.tile([BPC, T + 1], F32)   # [0 | cumsum] -> clamped
        ssum = spool.tile([BPC, 1], F32)
        rcol = spool.tile([BPC, 1], F32)
        fendf = spool.tile([BPC, 1], F32)
        nc.vector.memset(chatp[:, 0:1], 0.0)
        zcol = cpool.tile([128, 1], F32)
        nc.vector.memset(zcol[:], 0.0)
        ccur = [spool.tile([128, NT], F32, name=f"ccur{e}") for e in range(BPC)]
        cprev = [spool.tile([128, NT], F32, name=f"cprev{e}") for e in range(BPC)]

        xt = {}
        xb = {}
        for ex in range(BPC):
            # ---------------- phase A ----------------
            for q in range(NT // 4):
                xq = xpool.tile([128, 4, D], F32R, tag=f"x{q % 3}")
                src = x_in[ex, 512 * q:512 * (q + 1), :].rearrange(
                    "(i p) d -> p i d", p=128)
                nc.sync.dma_start(xq[:], src)
                xbq = xbpool.tile([128, 4, D], BF16, tag=f"xb{q}")
                srcb = xb_in[ex, 512 * q:512 * (q + 1), :].rearrange(
                    "(i p) d -> p i d", p=128)
                nc.scalar.dma_start(xbq[:], srcb)
                for j in range(4):
                    xt[(ex, 4 * q + j)] = xq[:, j, :]
                    xb[(ex, 4 * q + j)] = xbq[:, j, :]

            win = {}
            for w in range(4):
                wt = wpool.tile([128, NC4, 1026], F32R, tag="win")
                win[w] = wt
                if w == 0:
                    nc.vector.tensor_copy(
                        wt[:, :, 0:1],
                        zcol[:].unsqueeze(1).broadcast_to([128, NC4, 1]))
                if w == 3:
                    nc.vector.tensor_copy(
                        wt[:, :, 1025:1026],
                        zcol[:].unsqueeze(1).broadcast_to([128, NC4, 1]))

            sgex = spool.tile([1, T], F32, tag="sgex")
            for i in range(NT):
                xti = xt[(ex, i)]  # AP slice [128, 512]
                tp = ps_t.tile([128, NC4, 128], F32R, tag="tp")
                for c in range(NC4):
                    nc.tensor.transpose(tp[:, c, :], xti[:, 128 * c:128 * (c + 1)],
                                        ident[:])
                w, pos = i // 8, i % 8
                dst = win[w][:, :, 1 + 128 * pos: 1 + 128 * pos + 128]
                if i % 2 == 0:
                    nc.vector.tensor_copy(dst, tp[:])
                else:
                    nc.scalar.copy(dst, tp[:])
                if pos == 0 and i > 0:
                    nc.vector.tensor_copy(win[w - 1][:, :, 1025:1026], tp[:, :, 0:1])
                if pos == 7 and i < NT - 1:
                    nc.vector.tensor_copy(win[w + 1][:, :, 0:1], tp[:, :, 127:128])

            for w in range(NW):
                lg = ps_l.tile([128, 512], F32, tag="lg")
                for c in range(NC4):
                    pc = ps_c.tile([128, 512], F32, tag="pc")
                    for k in range(3):
                        m = c * 3 + k
                        nc.tensor.matmul(pc[:], diags[:, m * 128:(m + 1) * 128],
                                         win[w // 2][:, c, 512 * (w % 2) + k:512 * (w % 2) + k + 512],
                                         start=(k == 0), stop=(k == 2))
                    h = hpool.tile([128, 512], F32R, tag="h")
                    if c % 2 == 0:
                        nc.scalar.activation(h[:], pc[:], AF.Relu,
                                             bias=biasc[:, c:c + 1], scale=1.0)
                    else:
                        nc.vector.tensor_scalar(h[:], pc[:], biasc[:, c:c + 1], 0.0,
                                                AL.add, AL.max)
                    nc.tensor.matmul(lg[0:1, :], linw[:, c:c + 1], h[:],
                                     start=(c == 0), stop=(c == NC4 - 1))
                nc.scalar.activation(sgex[0:1, 512 * w:512 * (w + 1)],
                                     lg[0:1, :], AF.Sigmoid)
            nc.gpsimd.dma_start(sig2[ex:ex + 1, :], sgex[0:1, :])

        # ------------- rescale (needs all sigmoids) -------------
        nc.vector.tensor_tensor(sig2[:], sig2[:], mask2[:], AL.mult)
        nc.vector.tensor_reduce(ssum[:], sig2[:], axis=mybir.AxisListType.X,
                                op=AL.add)
        nc.gpsimd.dma_start(tnh_o[:], ssum[:])
        nc.vector.reciprocal(rcol[:], ssum[:])
        nc.vector.tensor_tensor(rcol[:], rcol[:], tgt2[:], AL.mult)
        nc.vector.tensor_scalar_mul(sig2[:], sig2[:], rcol[:, 0:1])  # alphas
        nc.gpsimd.dma_start(alphas_o[:], sig2[:])

        # ------------- chunked scan chain -------------
        KW = 512
        prev_fi = None
        prev_i1 = None
        for w in range(NW):
            lo, hi = KW * w, KW * (w + 1)
            a_w = sig2[:, lo:hi]
            # C0 chunk into chatp[:, 1+lo : 1+hi]
            init_c = 0.0 if w == 0 else chatp[:, lo:lo + 1]
            nc.vector.tensor_tensor_scan(chatp[:, 1 + lo:1 + hi], a_w, a_w,
                                         init_c, AL.add, AL.bypass)
            # floor guess chunk: fi [BPC, 513] i32, col0 = carry
            fi = kpool.tile([BPC, KW + 1], I32, tag="fi")
            if w == 0:
                nc.vector.memset(fi[:, 0:1], 0)
            else:
                nc.vector.tensor_copy(fi[:, 0:1], prev_fi[:, KW:KW + 1])
            nc.vector.tensor_scalar(fi[:, 1:KW + 1], chatp[:, 1 + lo:1 + hi],
                                    0.5, 0.0, AL.subtract, AL.bypass)
            d1 = kpool.tile([BPC, KW], F32, tag="d1")
            nc.vector.tensor_tensor(d1[:], fi[:, 1:KW + 1], fi[:, 0:KW], AL.is_gt)
            # CIF scan chunk
            i1 = kpool.tile([BPC, KW], F32, tag="i1")
            init_i = 0.0 if w == 0 else prev_i1[:, KW - 1:KW]
            nc.vector.tensor_tensor_scan(i1[:], a_w, d1[:], init_i,
                                         AL.add, AL.subtract)
            fr = kpool.tile([BPC, KW], F32, tag="fr")
            nc.vector.tensor_tensor(fr[:], i1[:], d1[:], AL.add)
            nc.gpsimd.dma_start(fires_o[:, lo:hi], fr[:])
            prev_fi, prev_i1 = fi, i1

        nc.vector.tensor_copy(fendf[:], prev_fi[:, KW:KW + 1])
        nc.vector.tensor_scalar(chatp[:], chatp[:], fendf[:, 0:1], 0.0,
                                AL.min, AL.bypass)

        # ------------- W build + phase D -------------
        for ex in range(BPC):
            nc.gpsimd.dma_start(cscr[ex:ex + 1, :], chatp[ex:ex + 1, :])
        for ex in range(BPC):
            src = cscr[ex, 1:T + 1].rearrange("(k p) -> p k", p=128)
            nc.gpsimd.dma_start(ccur[ex][:], src)
            srcp = cscr[ex, 0:T].rearrange("(k p) -> p k", p=128)
            nc.gpsimd.dma_start(cprev[ex][:], srcp)

        NQ = 4  # tiles per W block
        for ex in range(BPC):
            acp = ps_a.tile([128, D], F32, tag="acp")
            for q in range(NT // NQ):
                k0 = q * NQ
                zc = wbpool.tile([128, NQ, 128], F32, tag="zc")
                zp = wbpool.tile([128, NQ, 128], F32, tag="zp")
                wtl = wbpool.tile([128, NQ, 128], BF16, tag="wt")
                cur_bc = ccur[ex][:, k0:k0 + NQ].unsqueeze(2).broadcast_to(
                    [128, NQ, 128])
                prev_bc = cprev[ex][:, k0:k0 + NQ].unsqueeze(2).broadcast_to(
                    [128, NQ, 128])
                nj_bc = negj[:].unsqueeze(1).broadcast_to([128, NQ, 128])
                nc.vector.tensor_tensor(zc[:], cur_bc, nj_bc, AL.add)
                nc.vector.tensor_scalar(zc[:], zc[:], 1.0, 0.0, AL.min, AL.max)
                nc.gpsimd.tensor_tensor(zp[:], prev_bc, nj_bc, AL.add)
                nc.gpsimd.tensor_scalar(zp[:], zp[:], 1.0, 0.0, AL.min, AL.max)
                nc.vector.tensor_tensor(wtl[:], zc[:], zp[:], AL.subtract)
                for j in range(NQ):
                    i = k0 + j
                    nc.tensor.matmul(acp[:], wtl[:, j, :], xb[(ex, i)],
                                     start=(i == 0), stop=(i == NT - 1))
            acs = spool.tile([128, D], F32, tag="acs")
            nc.vector.tensor_copy(acs[:], acp[:])
            nc.gpsimd.dma_start(ac_o[ex, :, :], acs[:])

    nc.compile()
    _NC_CACHE["nc"] = nc
    return nc


def _consts(conv_w, conv_b, lin_w):
    conv_w = np.asarray(conv_w, np.float32)   # [D, 1, 3]
    conv_b = np.asarray(conv_b, np.float32)   # [D]
    lin_w = np.asarray(lin_w, np.float32)     # [1, D]
    w = conv_w[:, 0, :].copy()
    w[:, 1] += 1.0                            # fold residual
    diags = np.zeros((128, 12 * 128), np.float32)
    for c in range(NC4):
        for k in range(3):
            m = c * 3 + k
            blk = diags[:, m * 128:(m + 1) * 128]
            np.fill_diagonal(blk, w[c * 128:(c + 1) * 128, k])
    return dict(
        diags=diags,
        ident=np.eye(128, dtype=np.float32),
        linw=np.ascontiguousarray(lin_w[0].reshape(NC4, 128).T),
        biasc=np.ascontiguousarray(conv_b.reshape(NC4, 128).T),
        negj=np.ascontiguousarray(
            np.broadcast_to(-np.arange(128, dtype=np.float32), (128, 128))),
    )


def kernel(encoder_out, encoder_out_length, target_label_length,
           conv_w, conv_b, lin_w, lin_b, max_label_len):
    x = np.ascontiguousarray(np.asarray(encoder_out, np.float32))
    enc_len = np.asarray(encoder_out_length, np.int32)
    tgt = np.asarray(target_label_length, np.int32)
    assert float(np.abs(np.asarray(lin_b)).max()) == 0.0, "lin_b != 0 unsupported"
    consts = _consts(conv_w, conv_b, lin_w)

    ar = np.arange(T, dtype=np.int32)
    in_maps = []
    for c in range(NCORES):
        sl = slice(c * BPC, (c + 1) * BPC)
        m = dict(consts)
        m["x"] = x[sl]
        m["xb"] = x[sl].astype(ml_dtypes.bfloat16)
        m["mask"] = (ar[None, :] < enc_len[sl, None]).astype(np.float32)
        m["tgt"] = tgt[sl, None].astype(np.float32)
        in_maps.append(m)

    nc = _build_nc()
    res = run_bass_kernel_spmd(nc, in_maps, core_ids=list(range(NCORES))).results

    acoustic = np.concatenate([r["acoustic"] for r in res], 0)
    fires = np.concatenate([r["fires"] for r in res], 0)
    alphas = np.concatenate([r["alphas"] for r in res], 0)
    tnh = np.concatenate([r["tnh"] for r in res], 0)[:, 0]
    return acoustic, fires, tnh, alphas
